# revision 11
# baseline (speedup 1.0000x reference)
"""Multi-scale deformable attention (nearest sampling, sum over points) on
8 Trainium2 NeuronCores via Bass/Tile — DMA-gather edition.

v2 design: instead of the GpSimd ap_gather ucode (~50+ cycles per index,
read-command latency bound), samples are fetched with dma_gather: the Q7
cores generate one 256-byte DMA descriptor per (query, head, sample) and
the 16 SDMA engines execute the gather straight from HBM. Value is
pre-packed on host to bf16 [phase, chunk(2), key, 128ch] so a descriptor
fetches one 4-head channel-chunk of one key; the chunk offset is folded
into the per-partition index constants. Each 128-query block issues two
8192-index dma_gather calls (one per channel chunk) rotated across the 4
SWDGE queues — queue rotation is the key throughput lever: each queue's
descriptor ring only holds ~one call, so a single queue serializes on
ring drain (~85us/call) while four queues overlap transfers (~17us/call).

Per 128-query block:
  PE transposes the (x, y) sampling planes to [(head,level,point), query],
  DVE computes nearest-neighbor gather indices with a rint()-exact fp32
  chain, PE re-transposes (with a permutation matrix) into the wrapped
  int16 index layout dma_gather expects, two dma_gathers fetch
  2 x 8192 x 256B, and DVE segment-reduces the 16 samples per (query,
  head) directly into the [query, 256ch] output tile.

No cross-core communication; inputs/outputs are sharded/assembled on host.
"""
import numpy as np
import ml_dtypes

SPATIAL = [(64, 176), (32, 88), (16, 44), (8, 22)]
LVL_OFF = [0, 11264, 14080, 14784]
NKEY = 14960
BS, NQ = 6, 40000
QPP, PHASES, QB = 10000, 3, 128
N_CORES = 8
MAGIC = 12582912.0  # 1.5 * 2**23 : float32 round-to-nearest-even bias
NIDX = QB * 128     # indices per gather call: 128 q x 8 heads x 16 samples
TIME_REPS = 4       # unroll factor of the timing-variant program

_CACHE = {}


def _make_consts():
    """Per-partition constants in the pre-fold (sampling) partition order
    e = head*16 + level*4 + point, so level = (e%16)//4, chunk = e//64."""
    c = np.zeros((128, 8), np.float32)
    for p in range(128):
        ch = p // 64
        l = (p % 16) // 4
        h_l, w_l = SPATIAL[l]
        off = LVL_OFF[l] + ch * NKEY
        c[p, 0] = w_l
        c[p, 1] = h_l
        c[p, 2] = MAGIC - off
        c[p, 3] = off + w_l - 1
        c[p, 4] = h_l - 1
    return c


def _make_pm():
    """PM so that transpose(idxf_cols, PM) lands partition e at column P(e):
    e = h*16 + s  ->  P = (h//4)*64 + s*4 + (h%4)."""
    pm = np.zeros((128, 128), np.float32)
    for h in range(8):
        for s in range(16):
            e = h * 16 + s
            P = (h // 4) * 64 + s * 4 + (h % 4)
            pm[e, P] = 1.0
    return pm


def _build_program(qpp, phases, qb, reps=1):
    """reps > 1 unrolls the whole kernel body `reps` times (idempotent
    rewrites of the same outputs) — used to measure per-iteration device
    time as a slope, cancelling the ~70ms axon dispatch overhead."""
    from concourse import bacc, tile, mybir, library_config

    F32 = mybir.dt.float32
    I16 = mybir.dt.int16
    BF16 = mybir.dt.bfloat16
    A = mybir.AluOpType
    nblk = (qpp + qb - 1) // qb

    nc = bacc.Bacc("TRN2", target_bir_lowering=False, debug=False,
                   num_swdge_queues=4)
    val16 = nc.dram_tensor("val16", [phases, 2 * NKEY, 128], BF16,
                           kind="ExternalInput")
    samp3 = nc.dram_tensor("samp3", [phases, qpp, 256], F32,
                           kind="ExternalInput")
    consts = nc.dram_tensor("consts", [128, 8], F32, kind="ExternalInput")
    pm_in = nc.dram_tensor("pm", [128, 128], F32, kind="ExternalInput")
    ident_in = nc.dram_tensor("ident", [128, 128], F32, kind="ExternalInput")
    out_ext = nc.dram_tensor("out", [phases * qpp, 256], F32,
                             kind="ExternalOutput")

    with tile.TileContext(nc) as tc:
        with tc.tile_pool(name="cst", bufs=1) as cstp, \
             tc.tile_pool(name="sraw", bufs=6) as srawp, \
             tc.tile_pool(name="xy", bufs=6) as xyp, \
             tc.tile_pool(name="idxt", bufs=6) as idxtp, \
             tc.tile_pool(name="g", bufs=4) as gp, \
             tc.tile_pool(name="ost", bufs=4) as ostp, \
             tc.tile_pool(name="psxy", bufs=3, space="PSUM") as psxy, \
             tc.tile_pool(name="pst", bufs=2, space="PSUM") as pst:

            cst = cstp.tile([128, 8], F32, tag="cst")
            pm = cstp.tile([128, 128], F32, tag="pm")
            idn = cstp.tile([128, 128], F32, tag="idn")
            nc.sync.dma_start(out=cst[:], in_=consts[:])
            nc.sync.dma_start(out=pm[:], in_=pm_in[:])
            nc.sync.dma_start(out=idn[:], in_=ident_in[:])
            W = cst[:, 0:1]
            Hh = cst[:, 1:2]
            XC = cst[:, 2:3]
            XHI = cst[:, 3:4]
            YHI = cst[:, 4:5]

            nc.gpsimd.load_library(library_config.mlp)

            for ph in [p for _ in range(reps) for p in range(phases)]:
                src = val16[ph]
                for blk in range(nblk):
                    q0 = min(blk * qb, qpp - qb)
                    sraw = srawp.tile([128, 256], F32, tag="sraw")
                    nc.sync.dma_start(out=sraw[:],
                                      in_=samp3[ph, q0:q0 + qb, :])
                    # x and y transposes share one PSUM bank tile: frees a
                    # bank so bufs=3 lets the index chain run 3 blocks ahead
                    xyps = psxy.tile([128, 256], F32, tag="xyps")
                    xps = xyps[:, 0:128]
                    yps = xyps[:, 128:256]
                    sv = sraw[:].rearrange("q (e t) -> q e t", t=2)
                    nc.tensor.matmul(xps, sv[:, :, 0], idn[:],
                                     is_transpose=True,
                                     skip_group_check=True)
                    nc.tensor.matmul(yps, sv[:, :, 1], idn[:],
                                     is_transpose=True,
                                     skip_group_check=True)
                    xf = xyp.tile([128, qb], F32, tag="xf")
                    yf = xyp.tile([128, qb], F32, tag="yf")
                    idxf = xyp.tile([128, qb], F32, tag="idxf")
                    nc.vector.tensor_scalar(out=xf[:], in0=xps,
                                            scalar1=W, scalar2=-0.5,
                                            op0=A.mult, op1=A.add)
                    nc.vector.tensor_scalar(out=xf[:], in0=xf[:],
                                            scalar1=MAGIC, scalar2=XC,
                                            op0=A.add, op1=A.subtract)
                    nc.vector.tensor_scalar(out=xf[:], in0=xf[:],
                                            scalar1=XHI, scalar2=None,
                                            op0=A.min)
                    nc.vector.tensor_scalar(out=yf[:], in0=yps,
                                            scalar1=Hh, scalar2=-0.5,
                                            op0=A.mult, op1=A.add)
                    nc.vector.tensor_scalar(out=yf[:], in0=yf[:],
                                            scalar1=MAGIC, scalar2=MAGIC,
                                            op0=A.add, op1=A.subtract)
                    nc.vector.tensor_scalar(out=yf[:], in0=yf[:],
                                            scalar1=YHI, scalar2=None,
                                            op0=A.min)
                    nc.vector.scalar_tensor_tensor(out=idxf[:], in0=yf[:],
                                                   scalar=W, in1=xf[:],
                                                   op0=A.mult, op1=A.add)
                    # fold [e, q] -> wrapped idx layout [q%16, P(e)*8 + q//16]
                    tall = pst.tile([16, 1024], F32, tag="tall")
                    for g in range(8):
                        nc.tensor.transpose(tall[:, g * 128:(g + 1) * 128],
                                            idxf[:, g * 16:(g + 1) * 16],
                                            pm[:])
                    # num_idxs is capped by the 64KB-64 Q7 scratch: one
                    # 8192-index dma_gather per 4-head chunk (2 per block)
                    ost = ostp.tile([128, 256], F32, tag="ost")
                    tv = tall[:].rearrange("p (g c P) -> p c P g", g=8, c=2)
                    for c in range(2):
                        # the wrapped index list must be present in the 32
                        # partitions of whichever Q7 pair runs the queue;
                        # engines can't write past partition 0-15, so
                        # log-replicate to all 128 partitions with DMAs
                        idxt = idxtp.tile([128, 512], I16, tag=f"idxt{c}")
                        nc.scalar.copy(
                            out=idxt[0:16, :].rearrange(
                                "p (P g) -> p P g", g=8),
                            in_=tv[:, c])
                        nc.sync.dma_start(out=idxt[16:32, :],
                                          in_=idxt[0:16, :])
                        nc.sync.dma_start(out=idxt[32:64, :],
                                          in_=idxt[0:32, :])
                        nc.sync.dma_start(out=idxt[64:128, :],
                                          in_=idxt[0:64, :])
                        gout = gp.tile([128, NIDX // 2], BF16, tag=f"g{c}")
                        # two 4096-idx calls per chunk: finer grain keeps all
                        # 4 SWDGE queues fed (measured 2.24 vs 2.48 ns/desc)
                        nq4 = NIDX // 4
                        for m in range(2):
                            nc.gpsimd.dma_gather(
                                out_ap=gout[:, m * nq4:(m + 1) * nq4]
                                .rearrange("p (i e) -> p i e", e=128),
                                in_ap=src,
                                idxs_ap=idxt[:, m * (nq4 // 16):
                                             (m + 1) * (nq4 // 16)],
                                num_idxs=nq4, num_idxs_reg=nq4,
                                elem_size=128, single_packet=False,
                                queue_num=(c * 2 + m + blk) % 4)
                        gv = gout[:].rearrange("p (s h e) -> p s h e",
                                               s=16, h=4)
                        for h in range(4):
                            seg = gv[:, :, h, 32 * h:32 * h + 32]
                            nc.vector.tensor_reduce(
                                out=ost[:, c * 128 + h * 32:
                                        c * 128 + (h + 1) * 32],
                                in_=seg.rearrange("p s e -> p e s"),
                                axis=mybir.AxisListType.X, op=A.add)
                    row0 = ph * qpp + q0
                    nc.sync.dma_start(out=out_ext[row0:row0 + qb, :],
                                      in_=ost[:])
    return nc


def _compile_spmd(nc, n_cores):
    """Compile-once runner based on concourse.bass2jax.run_bass_via_pjrt."""
    import jax
    from jax.sharding import Mesh, PartitionSpec, NamedSharding
    try:
        from jax.experimental.shard_map import shard_map
    except ImportError:
        from jax.shard_map import shard_map
    from concourse import mybir
    from concourse.bass2jax import (
        install_neuronx_cc_hook, _bass_exec_p, partition_id_tensor)

    install_neuronx_cc_hook()
    if not nc.is_finalized():
        nc.finalize()
    partition_name = (nc.partition_id_tensor.name
                      if nc.partition_id_tensor else None)

    in_names, out_names, out_avals, zero_outs = [], [], [], []
    for alloc in nc.m.functions[0].allocations:
        if not isinstance(alloc, mybir.MemoryLocationSet):
            continue
        name = alloc.memorylocations[0].name
        if alloc.kind == "ExternalInput":
            if name != partition_name:
                in_names.append(name)
        elif alloc.kind == "ExternalOutput":
            out_names.append(name)
            shape = tuple(alloc.tensor_shape)
            dtype = mybir.dt.np(alloc.dtype)
            out_avals.append(jax.core.ShapedArray(shape, dtype))
            zero_outs.append(np.zeros(shape, dtype))
    n_params = len(in_names)
    all_in_names = (in_names + out_names
                    + ([partition_name] if partition_name else []))

    def _body(*args):
        operands = list(args)
        if partition_name is not None:
            operands.append(partition_id_tensor())
        outs = _bass_exec_p.bind(
            *operands,
            out_avals=tuple(out_avals),
            in_names=tuple(all_in_names),
            out_names=tuple(out_names),
            lowering_input_output_aliases=(),
            sim_require_finite=True,
            sim_require_nnan=True,
            nc=nc,
        )
        return tuple(outs)

    devices = jax.devices()[:n_cores]
    mesh = Mesh(np.asarray(devices), ("core",))
    in_specs = (PartitionSpec("core"),) * (n_params + len(out_names))
    out_specs = (PartitionSpec("core"),) * len(out_names)
    sharded = jax.jit(
        shard_map(_body, mesh=mesh, in_specs=in_specs,
                  out_specs=out_specs, check_rep=False),
        keep_unused=True,
    )
    sh = NamedSharding(mesh, PartitionSpec("core"))

    def prep(in_maps):
        staged = [
            jax.device_put(
                np.concatenate([m[name] for m in in_maps], axis=0), sh)
            for name in in_names
        ]
        staged += [
            jax.device_put(np.concatenate([z] * n_cores, axis=0), sh)
            for z in zero_outs
        ]
        return staged

    def run(staged):
        return sharded(*staged)
    return run, prep, in_names, out_names


def _get_compiled():
    if "run" not in _CACHE:
        nc = _build_program(QPP, PHASES, QB)
        run, prep, in_names, out_names = _compile_spmd(nc, N_CORES)
        _CACHE.update(run=run, prep=prep, in_names=in_names,
                      out_names=out_names)
    return _CACHE


def _get_compiled_reps():
    """Timing variant: same kernel unrolled TIME_REPS times."""
    if "run_r" not in _CACHE:
        nc = _build_program(QPP, PHASES, QB, reps=TIME_REPS)
        run_r, prep_r, _, _ = _compile_spmd(nc, N_CORES)
        _CACHE.update(run_r=run_r)
    return _CACHE


def _shard_inputs(value, sampling_locations):
    vbf = value.reshape(BS, NKEY, 256).astype(ml_dtypes.bfloat16)
    sflat = np.ascontiguousarray(
        sampling_locations.reshape(BS, NQ, 256))
    consts = _make_consts()
    pm = _make_pm()
    ident = np.eye(128, dtype=np.float32)
    in_maps = []
    for c in range(N_CORES):
        v16 = np.empty((PHASES, 2 * NKEY, 128), ml_dtypes.bfloat16)
        s3 = np.empty((PHASES, QPP, 256), np.float32)
        for j in range(PHASES):
            g = c * PHASES + j
            b = (g * QPP) // NQ
            q0 = (g * QPP) % NQ
            v16[j, :NKEY] = vbf[b, :, 0:128]
            v16[j, NKEY:] = vbf[b, :, 128:256]
            s3[j] = sflat[b, q0:q0 + QPP]
        in_maps.append({"val16": v16, "samp3": s3, "consts": consts,
                        "pm": pm, "ident": ident})
    return in_maps


def kernel(value, value_spatial_shapes, sampling_locations):
    import jax
    value = np.asarray(value, np.float32)
    sampling_locations = np.asarray(sampling_locations, np.float32)
    cc = _get_compiled()
    in_maps = _shard_inputs(value, sampling_locations)
    staged = cc["prep"](in_maps)
    outs = cc["run"](staged)
    jax.block_until_ready(outs)
    full = np.asarray(outs[0])                 # (8*30000, 256)
    return np.ascontiguousarray(full.reshape(BS, NQ, 256))



# revision 12
# speedup vs baseline: 1.4904x; 1.4904x over previous
"""Multi-scale deformable attention (nearest sampling, sum over points) on
8 Trainium2 NeuronCores via Bass/Tile — DMA-gather edition.

v2 design: instead of the GpSimd ap_gather ucode (~50+ cycles per index,
read-command latency bound), samples are fetched with dma_gather: the Q7
cores generate one 256-byte DMA descriptor per (query, head, sample) and
the 16 SDMA engines execute the gather straight from HBM. Value is
pre-packed on host to bf16 [phase, chunk(2), key, 128ch] so a descriptor
fetches one 4-head channel-chunk of one key; the chunk offset is folded
into the per-partition index constants. Each 128-query block issues two
8192-index dma_gather calls (one per channel chunk) rotated across the 4
SWDGE queues — queue rotation is the key throughput lever: each queue's
descriptor ring only holds ~one call, so a single queue serializes on
ring drain (~85us/call) while four queues overlap transfers (~17us/call).

Per 128-query block:
  PE transposes the (x, y) sampling planes to [(head,level,point), query],
  DVE computes nearest-neighbor gather indices with a rint()-exact fp32
  chain, PE re-transposes (with a permutation matrix) into the wrapped
  int16 index layout dma_gather expects, two dma_gathers fetch
  2 x 8192 x 256B, and DVE segment-reduces the 16 samples per (query,
  head) directly into the [query, 256ch] output tile.

No cross-core communication; inputs/outputs are sharded/assembled on host.
"""
import numpy as np
import ml_dtypes

SPATIAL = [(64, 176), (32, 88), (16, 44), (8, 22)]
LVL_OFF = [0, 11264, 14080, 14784]
NKEY = 14960
BS, NQ = 6, 40000
QPP, PHASES, QB = 10000, 3, 128
N_CORES = 8
MAGIC = 12582912.0  # 1.5 * 2**23 : float32 round-to-nearest-even bias
NIDX = QB * 128     # indices per gather call: 128 q x 8 heads x 16 samples
TIME_REPS = 4       # unroll factor of the timing-variant program

_CACHE = {}


def _make_consts():
    """Per-partition constants in the pre-fold (sampling) partition order
    e = head*16 + level*4 + point, so level = (e%16)//4, chunk = e//64."""
    c = np.zeros((128, 8), np.float32)
    for p in range(128):
        ch = p // 64
        l = (p % 16) // 4
        h_l, w_l = SPATIAL[l]
        off = LVL_OFF[l] + ch * NKEY
        c[p, 0] = w_l
        c[p, 1] = h_l
        c[p, 2] = MAGIC - off
        c[p, 3] = off + w_l - 1
        c[p, 4] = h_l - 1
    return c


def _make_pm():
    """PM so that transpose(idxf_cols, PM) lands partition e at column P(e):
    e = h*16 + s  ->  P = (h//4)*64 + s*4 + (h%4)."""
    pm = np.zeros((128, 128), np.float32)
    for h in range(8):
        for s in range(16):
            e = h * 16 + s
            P = (h // 4) * 64 + s * 4 + (h % 4)
            pm[e, P] = 1.0
    return pm


def _build_program(qpp, phases, qb, reps=1):
    """reps > 1 unrolls the whole kernel body `reps` times (idempotent
    rewrites of the same outputs) — used to measure per-iteration device
    time as a slope, cancelling the ~70ms axon dispatch overhead."""
    from concourse import bacc, tile, mybir, library_config

    F32 = mybir.dt.float32
    I16 = mybir.dt.int16
    BF16 = mybir.dt.bfloat16
    A = mybir.AluOpType
    nblk = (qpp + qb - 1) // qb

    nc = bacc.Bacc("TRN2", target_bir_lowering=False, debug=False,
                   num_swdge_queues=4)
    val16 = nc.dram_tensor("val16", [phases, 2 * NKEY, 128], BF16,
                           kind="ExternalInput")
    samp3 = nc.dram_tensor("samp3", [phases, qpp, 256], F32,
                           kind="ExternalInput")
    consts = nc.dram_tensor("consts", [128, 8], F32, kind="ExternalInput")
    pm_in = nc.dram_tensor("pm", [128, 128], F32, kind="ExternalInput")
    ident_in = nc.dram_tensor("ident", [128, 128], F32, kind="ExternalInput")
    out_ext = nc.dram_tensor("out", [phases * qpp, 256], F32,
                             kind="ExternalOutput")

    with tile.TileContext(nc) as tc:
        with tc.tile_pool(name="cst", bufs=1) as cstp, \
             tc.tile_pool(name="sraw", bufs=4) as srawp, \
             tc.tile_pool(name="xy", bufs=4) as xyp, \
             tc.tile_pool(name="idxt", bufs=4) as idxtp, \
             tc.tile_pool(name="g", bufs=4) as gp, \
             tc.tile_pool(name="ost", bufs=4) as ostp, \
             tc.tile_pool(name="psxy", bufs=2, space="PSUM") as psxy, \
             tc.tile_pool(name="pst", bufs=2, space="PSUM") as pst:

            cst = cstp.tile([128, 8], F32, tag="cst")
            pm = cstp.tile([128, 128], F32, tag="pm")
            idn = cstp.tile([128, 128], F32, tag="idn")
            nc.sync.dma_start(out=cst[:], in_=consts[:])
            nc.sync.dma_start(out=pm[:], in_=pm_in[:])
            nc.sync.dma_start(out=idn[:], in_=ident_in[:])
            W = cst[:, 0:1]
            Hh = cst[:, 1:2]
            XC = cst[:, 2:3]
            XHI = cst[:, 3:4]
            YHI = cst[:, 4:5]

            nc.gpsimd.load_library(library_config.mlp)

            for ph in [p for _ in range(reps) for p in range(phases)]:
                src = val16[ph]
                for blk in range(nblk):
                    q0 = min(blk * qb, qpp - qb)
                    sraw = srawp.tile([128, 256], F32, tag="sraw")
                    nc.sync.dma_start(out=sraw[:],
                                      in_=samp3[ph, q0:q0 + qb, :])
                    xps = psxy.tile([128, 128], F32, tag="xps")
                    yps = psxy.tile([128, 128], F32, tag="yps")
                    sv = sraw[:].rearrange("q (e t) -> q e t", t=2)
                    nc.tensor.transpose(xps[:], sv[:, :, 0], idn[:])
                    nc.tensor.transpose(yps[:], sv[:, :, 1], idn[:])
                    xf = xyp.tile([128, qb], F32, tag="xf")
                    yf = xyp.tile([128, qb], F32, tag="yf")
                    idxf = xyp.tile([128, qb], F32, tag="idxf")
                    nc.vector.tensor_scalar(out=xf[:], in0=xps[:],
                                            scalar1=W, scalar2=-0.5,
                                            op0=A.mult, op1=A.add)
                    nc.vector.tensor_scalar(out=xf[:], in0=xf[:],
                                            scalar1=MAGIC, scalar2=XC,
                                            op0=A.add, op1=A.subtract)
                    nc.vector.tensor_scalar(out=xf[:], in0=xf[:],
                                            scalar1=XHI, scalar2=None,
                                            op0=A.min)
                    nc.vector.tensor_scalar(out=yf[:], in0=yps[:],
                                            scalar1=Hh, scalar2=-0.5,
                                            op0=A.mult, op1=A.add)
                    nc.vector.tensor_scalar(out=yf[:], in0=yf[:],
                                            scalar1=MAGIC, scalar2=MAGIC,
                                            op0=A.add, op1=A.subtract)
                    nc.vector.tensor_scalar(out=yf[:], in0=yf[:],
                                            scalar1=YHI, scalar2=None,
                                            op0=A.min)
                    nc.vector.scalar_tensor_tensor(out=idxf[:], in0=yf[:],
                                                   scalar=W, in1=xf[:],
                                                   op0=A.mult, op1=A.add)
                    # fold [e, q] -> wrapped idx layout [q%16, P(e)*8 + q//16]
                    tall = pst.tile([16, 1024], F32, tag="tall")
                    for g in range(8):
                        nc.tensor.transpose(tall[:, g * 128:(g + 1) * 128],
                                            idxf[:, g * 16:(g + 1) * 16],
                                            pm[:])
                    # num_idxs is capped by the 64KB-64 Q7 scratch: one
                    # 8192-index dma_gather per 4-head chunk (2 per block)
                    ost = ostp.tile([128, 256], F32, tag="ost")
                    tv = tall[:].rearrange("p (g c P) -> p c P g", g=8, c=2)
                    for c in range(2):
                        # the wrapped index list must be present in the 32
                        # partitions of whichever Q7 pair runs the queue;
                        # engines can't write past partition 0-15, so
                        # log-replicate to all 128 partitions with DMAs
                        idxt = idxtp.tile([128, 512], I16, tag=f"idxt{c}")
                        nc.scalar.copy(
                            out=idxt[0:16, :].rearrange(
                                "p (P g) -> p P g", g=8),
                            in_=tv[:, c])
                        nc.sync.dma_start(out=idxt[16:32, :],
                                          in_=idxt[0:16, :])
                        nc.sync.dma_start(out=idxt[32:64, :],
                                          in_=idxt[0:32, :])
                        nc.sync.dma_start(out=idxt[64:128, :],
                                          in_=idxt[0:64, :])
                        gout = gp.tile([128, NIDX // 2], BF16, tag=f"g{c}")
                        # two 4096-idx calls per chunk: finer grain keeps all
                        # 4 SWDGE queues fed (measured 2.24 vs 2.48 ns/desc)
                        nq4 = NIDX // 4
                        for m in range(2):
                            nc.gpsimd.dma_gather(
                                out_ap=gout[:, m * nq4:(m + 1) * nq4]
                                .rearrange("p (i e) -> p i e", e=128),
                                in_ap=src,
                                idxs_ap=idxt[:, m * (nq4 // 16):
                                             (m + 1) * (nq4 // 16)],
                                num_idxs=nq4, num_idxs_reg=nq4,
                                elem_size=128, single_packet=False,
                                queue_num=(c * 2 + m + blk) % 4)
                        gv = gout[:].rearrange("p (s h e) -> p s h e",
                                               s=16, h=4)
                        for h in range(4):
                            seg = gv[:, :, h, 32 * h:32 * h + 32]
                            nc.vector.tensor_reduce(
                                out=ost[:, c * 128 + h * 32:
                                        c * 128 + (h + 1) * 32],
                                in_=seg.rearrange("p s e -> p e s"),
                                axis=mybir.AxisListType.X, op=A.add)
                    row0 = ph * qpp + q0
                    nc.sync.dma_start(out=out_ext[row0:row0 + qb, :],
                                      in_=ost[:])
    return nc


def _compile_spmd(nc, n_cores):
    """Compile-once runner based on concourse.bass2jax.run_bass_via_pjrt."""
    import jax
    from jax.sharding import Mesh, PartitionSpec, NamedSharding
    try:
        from jax.experimental.shard_map import shard_map
    except ImportError:
        from jax.shard_map import shard_map
    from concourse import mybir
    from concourse.bass2jax import (
        install_neuronx_cc_hook, _bass_exec_p, partition_id_tensor)

    install_neuronx_cc_hook()
    if not nc.is_finalized():
        nc.finalize()
    partition_name = (nc.partition_id_tensor.name
                      if nc.partition_id_tensor else None)

    in_names, out_names, out_avals, zero_outs = [], [], [], []
    for alloc in nc.m.functions[0].allocations:
        if not isinstance(alloc, mybir.MemoryLocationSet):
            continue
        name = alloc.memorylocations[0].name
        if alloc.kind == "ExternalInput":
            if name != partition_name:
                in_names.append(name)
        elif alloc.kind == "ExternalOutput":
            out_names.append(name)
            shape = tuple(alloc.tensor_shape)
            dtype = mybir.dt.np(alloc.dtype)
            out_avals.append(jax.core.ShapedArray(shape, dtype))
            zero_outs.append(np.zeros(shape, dtype))
    n_params = len(in_names)
    all_in_names = (in_names + out_names
                    + ([partition_name] if partition_name else []))

    def _body(*args):
        operands = list(args)
        if partition_name is not None:
            operands.append(partition_id_tensor())
        outs = _bass_exec_p.bind(
            *operands,
            out_avals=tuple(out_avals),
            in_names=tuple(all_in_names),
            out_names=tuple(out_names),
            lowering_input_output_aliases=(),
            sim_require_finite=True,
            sim_require_nnan=True,
            nc=nc,
        )
        return tuple(outs)

    devices = jax.devices()[:n_cores]
    mesh = Mesh(np.asarray(devices), ("core",))
    in_specs = (PartitionSpec("core"),) * (n_params + len(out_names))
    out_specs = (PartitionSpec("core"),) * len(out_names)
    sharded = jax.jit(
        shard_map(_body, mesh=mesh, in_specs=in_specs,
                  out_specs=out_specs, check_rep=False),
        keep_unused=True,
    )
    sh = NamedSharding(mesh, PartitionSpec("core"))

    def prep(in_maps):
        staged = [
            jax.device_put(
                np.concatenate([m[name] for m in in_maps], axis=0), sh)
            for name in in_names
        ]
        staged += [
            jax.device_put(np.concatenate([z] * n_cores, axis=0), sh)
            for z in zero_outs
        ]
        return staged

    def run(staged):
        return sharded(*staged)
    return run, prep, in_names, out_names


def _get_compiled():
    if "run" not in _CACHE:
        nc = _build_program(QPP, PHASES, QB)
        run, prep, in_names, out_names = _compile_spmd(nc, N_CORES)
        _CACHE.update(run=run, prep=prep, in_names=in_names,
                      out_names=out_names)
    return _CACHE


def _get_compiled_reps():
    """Timing variant: same kernel unrolled TIME_REPS times."""
    if "run_r" not in _CACHE:
        nc = _build_program(QPP, PHASES, QB, reps=TIME_REPS)
        run_r, prep_r, _, _ = _compile_spmd(nc, N_CORES)
        _CACHE.update(run_r=run_r)
    return _CACHE


def _shard_inputs(value, sampling_locations):
    vbf = value.reshape(BS, NKEY, 256).astype(ml_dtypes.bfloat16)
    sflat = np.ascontiguousarray(
        sampling_locations.reshape(BS, NQ, 256))
    consts = _make_consts()
    pm = _make_pm()
    ident = np.eye(128, dtype=np.float32)
    in_maps = []
    for c in range(N_CORES):
        v16 = np.empty((PHASES, 2 * NKEY, 128), ml_dtypes.bfloat16)
        s3 = np.empty((PHASES, QPP, 256), np.float32)
        for j in range(PHASES):
            g = c * PHASES + j
            b = (g * QPP) // NQ
            q0 = (g * QPP) % NQ
            v16[j, :NKEY] = vbf[b, :, 0:128]
            v16[j, NKEY:] = vbf[b, :, 128:256]
            s3[j] = sflat[b, q0:q0 + QPP]
        in_maps.append({"val16": v16, "samp3": s3, "consts": consts,
                        "pm": pm, "ident": ident})
    return in_maps


def kernel(value, value_spatial_shapes, sampling_locations):
    import jax
    value = np.asarray(value, np.float32)
    sampling_locations = np.asarray(sampling_locations, np.float32)
    cc = _get_compiled()
    in_maps = _shard_inputs(value, sampling_locations)
    staged = cc["prep"](in_maps)
    outs = cc["run"](staged)
    jax.block_until_ready(outs)
    full = np.asarray(outs[0])                 # (8*30000, 256)
    return np.ascontiguousarray(full.reshape(BS, NQ, 256))

